# revision 25
# baseline (speedup 1.0000x reference)
"""v8: Tensor-parallel attention on 8 TRN2 cores.

Trace-driven changes over v7 (1379us, TensorE 90.4% busy, idle 133us):
- wo load split across 4 DMA queues and issued right at phase-2 entry
  (was: 2 serialized issues on the gpsimd queue BEHIND the last AllGather
  -> 36us TensorE gap).
- Initial weight tiles spread over scalar/gpsimd/vector queues in
  consumption order (v,q,k x c4); x chunk 0 split in half across
  sync+vector so the first matmul starts ~10us earlier.
- cos/sin loaded ONCE as persistent bf16 tiles (was 512KB per chunk on
  the sync queue, 8MB total, serialized with x loads).
- Queue reassignment by producer/consumer: q/k transposes + v loads on
  gpsimd (idle), qkv stage-outs + cc_in writes on vector directly after
  their producing vector op (no semaphore-wait stalls in front of the
  exp chain on scalar; sync carries only x).
- Scores->PV software pipeline with one-block lookahead across the
  (head, j) sequence so PV never waits on the exp/mask chain.
"""

import math
import sys

import numpy as np

sys.path.insert(0, "/opt/trn_rl_repo")

import ml_dtypes  # noqa: E402

import concourse.bass as bass  # noqa: E402,F401
import concourse.mybir as mybir  # noqa: E402
from concourse import bacc, tile  # noqa: E402
from concourse.bass_utils import run_bass_kernel_spmd  # noqa: E402

B, S, D, H = 2, 2048, 4096, 32
HD = 128
NCORES = 8
HLOC = H // NCORES          # 4 heads per core
EL = HLOC * HD              # 512
T = B * S                   # 4096
P = 128
NT = T // P                 # 32
ND = D // P                 # 32
NB = S // P                 # 16 key blocks per batch
IC = 512                    # query-chunk width in attention
NIC = S // IC               # 4 query chunks per batch
SQ = S // NIC               # 512 tokens per staged quarter
SCALE = 1.0 / math.sqrt(HD)
BF = mybir.dt.bfloat16
F32 = mybir.dt.float32
TCH = 256
NCH_B = S // TCH            # x chunks per batch element (8)
NCH = T // TCH

_GRAPH_CACHE: dict[bytes, object] = {}
LAST_RESULTS = None


def _classify_mask(mask: np.ndarray):
    """Per 128x128 block: -2 all-zero, -1 fully masked (<=-1e8), else index
    into the list of unique blocks (converted to exp factors host-side)."""
    mb = mask.reshape(NB, P, NB, P).transpose(0, 2, 1, 3)
    kinds = np.empty((NB, NB), np.int64)
    uniq: dict[bytes, int] = {}
    blocks: list[np.ndarray] = []
    for i in range(NB):
        for j in range(NB):
            blk = mb[i, j]
            if not blk.any():
                kinds[i, j] = -2
            elif np.all(blk <= -1e8):
                kinds[i, j] = -1
            else:
                key = blk.tobytes()
                if key not in uniq:
                    uniq[key] = len(blocks)
                    blocks.append(np.ascontiguousarray(blk, np.float32))
                kinds[i, j] = uniq[key]
    return kinds, blocks


def _build(kinds: np.ndarray, n_blocks: int):
    nu = max(1, n_blocks)
    nc = bacc.Bacc("TRN2", target_bir_lowering=False, debug=False,
                   num_devices=NCORES)
    xt = nc.dram_tensor("xt", [NCH, P, ND * TCH], BF, kind="ExternalInput")
    wqt = nc.dram_tensor("wqt", [4, P, 8 * EL], BF, kind="ExternalInput")
    wkt = nc.dram_tensor("wkt", [4, P, 8 * EL], BF, kind="ExternalInput")
    wvt = nc.dram_tensor("wvt", [4, P, 8 * EL], BF, kind="ExternalInput")
    wot = nc.dram_tensor("wot", [P, ND * EL], BF, kind="ExternalInput")
    # cos/sin pre-tiled host-side to [P, (S//P)*(EL//2)] so the one-time
    # load is 128 fat 8KB descriptors instead of 8192 512B ones.
    cosr = nc.dram_tensor("cosr", [P, (S // P) * (EL // 2)], BF,
                          kind="ExternalInput")
    sinr = nc.dram_tensor("sinr", [P, (S // P) * (EL // 2)], BF,
                          kind="ExternalInput")
    mblk = nc.dram_tensor("mblk", [nu, P, P], BF, kind="ExternalInput")
    out = nc.dram_tensor("out", [T, EL], BF, kind="ExternalOutput")

    qtd = [[nc.dram_tensor(f"qnd{b}_{q}", [SQ, EL], BF) for q in range(NIC)]
           for b in range(B)]
    ktd = [[nc.dram_tensor(f"knd{b}_{q}", [SQ, EL], BF) for q in range(NIC)]
           for b in range(B)]
    # v staged partition-major: vd[p, n*EL+e] = v[n*128+p, e] so the
    # attention-side load is 4KB contiguous per partition.
    vd = [[nc.dram_tensor(f"vd{b}_{q}", [P, (SQ // P) * EL], BF)
           for q in range(NIC)] for b in range(B)]
    NCC = B * NIC
    cc_in = [nc.dram_tensor(f"cc_in{k}", [EL, IC], BF) for k in range(NCC)]
    cc_out = [nc.dram_tensor(f"cc_out{k}", [NCORES * EL, IC], BF,
                             addr_space="Shared") for k in range(NCC)]

    jlists = []
    for icq in range(NIC):
        jl = [j for j in range(NB)
              if any(kinds[4 * icq + bi, j] != -1 for bi in range(4))]
        assert jl, "fully-masked query chunk"
        jlists.append(jl)

    with tile.TileContext(nc) as tc:
        with (
            tc.tile_pool(name="const", bufs=1) as cpool,
            tc.tile_pool(name="apool", bufs=1) as apool,
            tc.tile_pool(name="vpool", bufs=1) as vpool,
            tc.tile_pool(name="ptpool", bufs=3) as ptpool,
            tc.tile_pool(name="accp", bufs=2) as accp,
            tc.tile_pool(name="rpool", bufs=1) as rpool,
            tc.tile_pool(name="opool", bufs=2) as opool,
            tc.tile_pool(name="spp", bufs=3, space="PSUM") as spp,
            tc.tile_pool(name="dnp", bufs=1, space="PSUM") as dnp,
            tc.tile_pool(name="otp", bufs=2, space="PSUM") as otp,
        ):
            ones_sb = cpool.tile([P, P], BF, name="ones_sb")
            nc.vector.memset(ones_sb, 1.0)
            mb_sb = cpool.tile([P, nu, P], BF, name="mb_sb")
            nc.scalar.dma_start(mb_sb, mblk.ap().rearrange("n p q -> p n q"))
            # cos/sin for all positions, persistent; loaded once on the
            # gpsimd SW-DGE, emitted mid-weight-stream (only the vector
            # rotary needs them, which has ~30us of slack at cold start).
            cs_all = cpool.tile([P, S // P, EL // 2], BF, name="cs_all")
            sn_all = cpool.tile([P, S // P, EL // 2], BF, name="sn_all")

            def load_cs_all():
                nc.gpsimd.dma_start(
                    cs_all,
                    cosr.ap().rearrange("p (n e) -> p n e", n=S // P))
                nc.gpsimd.dma_start(
                    sn_all,
                    sinr.ap().rearrange("p (n e) -> p n e", n=S // P))

            x_pre: dict[int, object] = {}

            def load_x(cg, xpool, nsplit=1):
                xp = xpool.tile([P, ND, TCH], BF, tag="x1")
                step = ND // nsplit
                for s in range(nsplit):
                    dsl = slice(s * step, (s + 1) * step)
                    nc.sync.dma_start(
                        xp[:, dsl, :],
                        xt.ap()[cg].rearrange("p (n t) -> p n t", t=TCH)
                        [:, dsl, :])
                return xp

            def proj_chunk(b, chb, w_parts, xpool, stg, rot, pp):
                cg = b * NCH_B + chb
                xp = x_pre.pop(cg) if cg in x_pre else load_x(cg, xpool)
                for tt in range(TCH // P):
                    s0 = chb * TCH + tt * P
                    cs_sb = cs_all[:, s0 // P, :]
                    sn_sb = sn_all[:, s0 // P, :]
                    tsl = slice(tt * P, (tt + 1) * P)
                    qq = s0 // SQ
                    r0 = s0 % SQ
                    for w_parts_i, dstd in ((0, None), (1, qtd), (2, ktd)):
                        wp = w_parts[w_parts_i]
                        ps = pp.tile([P, EL], F32, tag="pqkv")
                        for dt in range(ND):
                            nc.tensor.matmul(
                                ps, xp[:, dt, tsl], wp[dt // 8][:, dt % 8],
                                start=(dt == 0), stop=(dt == ND - 1))
                        if dstd is None:
                            v_stage = stg.tile([P, EL], BF, tag="vstg")
                            nc.any.tensor_copy(v_stage, ps)
                            nn = r0 // P
                            nc.sync.dma_start(
                                vd[b][qq].ap()[:, nn * EL:(nn + 1) * EL],
                                v_stage)
                            continue
                        qn = stg.tile([P, EL], BF, tag="qn")
                        pe = ps.rearrange("p (r two) -> p r two", two=2)
                        qe = qn.rearrange("p (r two) -> p r two", two=2)
                        t1 = rot.tile([P, EL // 2], F32, tag="t1")
                        t2 = rot.tile([P, EL // 2], F32, tag="t2")
                        nc.vector.tensor_mul(t1, pe[:, :, 0], cs_sb)
                        nc.vector.tensor_mul(t2, pe[:, :, 1], sn_sb)
                        nc.vector.tensor_sub(qe[:, :, 0], t1, t2)
                        nc.vector.tensor_mul(t1, pe[:, :, 0], sn_sb)
                        nc.vector.tensor_mul(t2, pe[:, :, 1], cs_sb)
                        nc.vector.tensor_add(qe[:, :, 1], t1, t2)
                        nc.sync.dma_start(
                            dstd[b][qq].ap()[r0:r0 + P, :], qn)

            def attn_state():
                return {"qk": [{} for _ in range(HLOC)], "vt": {}}

            def attn_chunk_group(b, icq, st, after_head=None):
                qk, vt = st["qk"], st["vt"]
                # q/k transposes stay on scalar (HWDGE-only op); v tile on
                # the (otherwise idle) gpsimd SW-DGE queue.
                for hh in range(HLOC):
                    qt = apool.tile([P, SQ], BF, tag=f"qq_{hh}")
                    kt = apool.tile([P, SQ], BF, tag=f"k{icq}_{hh}")
                    nc.scalar.dma_start_transpose(
                        qt, qtd[b][icq].ap()[:, hh * P:(hh + 1) * P])
                    nc.sync.dma_start_transpose(
                        kt, ktd[b][icq].ap()[:, hh * P:(hh + 1) * P])
                    qk[hh][f"q{icq}"] = qt
                    qk[hh][f"k{icq}"] = kt
                v_t = vpool.tile([P, NB // NIC, EL], BF, tag=f"v{icq}")
                nc.gpsimd.dma_start(
                    v_t,
                    vd[b][icq].ap().rearrange("p (n e) -> p n e", e=EL))
                vt[icq] = v_t

                def ksl(hh, j):
                    t = qk[hh][f"k{j // 4}"]
                    return t[:, (j % 4) * P:((j % 4) + 1) * P]

                def vsl(hh, j):
                    return vt[j // 4][:, j % 4, hh * P:(hh + 1) * P]

                jl = jlists[icq]

                def lead(hh, idx, j):
                    # leading fully-masked i-sub-blocks contribute 0:
                    # narrow ops to the live suffix (first j must stay
                    # full-width to init the psum accumulation group)
                    if idx == 0:
                        return 0
                    nlead = 0
                    for bi in range(4):
                        if kinds[4 * icq + bi, j] == -1:
                            nlead += 1
                        else:
                            break
                    return nlead * P

                # software pipeline with one-block lookahead: emit the
                # scores matmul for op i+1 before the PV matmul of op i so
                # TensorE never waits on the exp/mask chain.
                ops = [(hh, idx, j) for hh in range(HLOC)
                       for idx, j in enumerate(jl)]
                state = {}

                def emit_scores(hh, idx, j):
                    off = lead(hh, idx, j)
                    qslice = qk[hh][f"q{icq}"]
                    sps = spp.tile([P, IC], F32, tag="sps")
                    nc.tensor.matmul(
                        sps[:, off:], ksl(hh, j), qslice[:, off:],
                        start=True, stop=True)
                    pt = ptpool.tile([P, IC], BF, tag="pt")
                    nc.scalar.activation(
                        pt[:, off:], sps[:, off:],
                        mybir.ActivationFunctionType.Exp, scale=SCALE)
                    for bi in range(off // P, 4):
                        k = kinds[4 * icq + bi, j]
                        sl = slice(bi * P, (bi + 1) * P)
                        if k == -1:
                            nc.vector.memset(pt[:, sl], 0.0)
                        elif k >= 0:
                            nc.vector.tensor_mul(
                                pt[:, sl], pt[:, sl], mb_sb[:, k, :])
                    return pt, off

                def emit_pv(hh, idx, j, pt, off):
                    stt = idx == 0
                    sp = idx == len(jl) - 1
                    if stt:
                        acc = accp.tile([P, IC], F32, tag="acc")
                        state["acc"] = acc
                        nc.vector.tensor_copy(acc, pt)
                        ot_ps = otp.tile([P, IC], F32, tag="ot_ps")
                        state["ot_ps"] = ot_ps
                    else:
                        acc = state["acc"]
                        ot_ps = state["ot_ps"]
                        nc.vector.tensor_add(
                            acc[:, off:], acc[:, off:], pt[:, off:])
                    nc.tensor.matmul(
                        ot_ps[:, off:], vsl(hh, j), pt[:, off:],
                        start=stt, stop=sp)
                    if sp:
                        accb = ptpool.tile([P, IC], BF, tag="accb")
                        nc.vector.tensor_copy(accb, acc)
                        den_ps = dnp.tile([P, IC], F32, tag="den_ps")
                        nc.tensor.matmul(den_ps, ones_sb, accb,
                                         start=True, stop=True)
                        rec = rpool.tile([P, IC], F32, tag="rec")
                        nc.vector.reciprocal_approx_fast(rec, den_ps)
                        ot_sb = opool.tile([P, IC], BF, tag="ot_sb")
                        nc.vector.tensor_mul(ot_sb, ot_ps, rec)
                        # cc_in writes on gpsimd: naturally ordered right
                        # before this chunk's AllGather on the same queue,
                        # and their sem-waits never block the exp chain.
                        nc.gpsimd.dma_start(
                            cc_in[b * NIC + icq]
                            .ap()[hh * P:(hh + 1) * P, :], ot_sb)

                def pop_one(o, po):
                    emit_pv(*o, *po)
                    if after_head is not None and o[1] == len(jl) - 1:
                        after_head(o[0])

                pend: list = []
                for op in ops:
                    pt_off = emit_scores(*op)
                    pend.append((op, pt_off))
                    if len(pend) > 2:
                        pop_one(*pend.pop(0))
                for o, po in pend:
                    pop_one(o, po)

                k = b * NIC + icq
                nc.gpsimd.collective_compute(
                    "AllGather", mybir.AluOpType.bypass,
                    ins=[cc_in[k].ap().opt()],
                    outs=[cc_out[k].ap().opt()],
                    replica_groups=[list(range(NCORES))],
                )

            with (
                tc.tile_pool(name="wpool", bufs=1) as wpool,
                tc.tile_pool(name="xpool", bufs=2) as xpool,
                tc.tile_pool(name="stg", bufs=2) as stg,
                tc.tile_pool(name="rot", bufs=2) as rot,
                tc.tile_pool(name="pp", bufs=2, space="PSUM") as pp,
            ):
                # prefetch x for the first two chunks BEFORE the weight
                # tiles hit the sync queue, then spread the 12 weight
                # tiles over all three DMA queues in consumption order
                # (v,q,k x c4) balanced against each queue's other load
                # so the cold-start is aggregate-BW-bound.
                x_pre[0] = load_x(0, xpool, nsplit=4)
                x_pre[1] = load_x(1, xpool)
                w_parts = [[], [], []]
                weng = {(0, 0): nc.scalar, (0, 1): nc.gpsimd,
                        (0, 2): nc.scalar, (0, 3): nc.gpsimd,
                        (1, 0): nc.gpsimd, (1, 1): nc.scalar,
                        (1, 2): nc.gpsimd, (1, 3): nc.gpsimd,
                        (2, 0): nc.scalar, (2, 1): nc.scalar,
                        (2, 2): nc.sync, (2, 3): nc.sync}
                for wi, w_d in ((0, wvt), (1, wqt), (2, wkt)):
                    for c4 in range(4):
                        wp = wpool.tile([P, 8, EL], BF, name=f"w{wi}_{c4}")
                        src = w_d.ap()[c4].rearrange("p (n e) -> p n e",
                                                     e=EL)
                        if (wi, c4) == (0, 0):
                            # first-consumed tile: two halves so dt0-3
                            # land ~5us earlier
                            nc.scalar.dma_start(wp[:, :4, :], src[:, :4, :])
                            nc.scalar.dma_start(wp[:, 4:, :], src[:, 4:, :])
                        else:
                            weng[(wi, c4)].dma_start(wp, src)
                        w_parts[wi].append(wp)
                        if (wi, c4) == (1, 0):
                            load_cs_all()
                st_last = None
                for b in range(B):
                    st = attn_state()
                    for chb in range(NCH_B):
                        proj_chunk(b, chb, w_parts, xpool, stg, rot, pp)
                        if chb % 2 == 1:
                            icq = (chb - 1) // 2
                            if b == B - 1 and icq == NIC - 1:
                                st_last = st  # deferred into phase 2
                            else:
                                attn_chunk_group(b, icq, st)

            with (
                tc.tile_pool(name="wop", bufs=1) as wop,
                tc.tile_pool(name="ccp", bufs=2) as ccp,
                tc.tile_pool(name="obp", bufs=2) as obp,
                tc.tile_pool(name="wpp", bufs=2, space="PSUM") as wpp,
            ):
                wo_sb = wop.tile([P, ND, EL], BF, name="wo_sb")
                woeng = [nc.sync, nc.scalar, nc.gpsimd, nc.sync]
                for q in range(4):
                    hsl = slice(q * (ND // 4) * EL, (q + 1) * (ND // 4) * EL)
                    woeng[q].dma_start(
                        wo_sb[:, q * (ND // 4):(q + 1) * (ND // 4), :],
                        wot.ap()[:, hsl]
                        .rearrange("p (n e) -> p n e", e=EL))
                cceng = [nc.sync, nc.scalar, nc.gpsimd, nc.scalar]

                def load_cct(k):
                    cct = ccp.tile([P, ND, IC], BF, tag="cct")
                    # parallel issues -> hw DMA queues chew descriptors
                    # concurrently (one 4MB issue serializes for ~50us)
                    for q in range(4):
                        cceng[q].dma_start(
                            cct[:, q * 8:(q + 1) * 8, :],
                            cc_out[k].ap()[q * 8 * P:(q + 1) * 8 * P, :]
                            .rearrange("(n p) t -> p n t", p=P))
                    return cct

                def outproj_tt(k, tt, cct):
                    g = k * (IC // P) + tt
                    ops = wpp.tile([P, EL], F32, tag="ops")
                    for ct in range(ND):
                        nc.tensor.matmul(
                            ops, cct[:, ct, tt * P:(tt + 1) * P],
                            wo_sb[:, ct],
                            start=(ct == 0), stop=(ct == ND - 1))
                    ob = obp.tile([P, EL], BF, tag="ob")
                    nc.any.tensor_copy(ob, ops)
                    nc.scalar.dma_start(
                        out.ap()[g * P:(g + 1) * P, :], ob)

                # the last attention group (b1,q3) runs HERE, its heads
                # interleaved with out-proj chunk 0's token tiles so the
                # exp/vector chain of the final group hides under out-proj
                # matmuls instead of stalling TensorE.
                cct0 = load_cct(0)
                attn_chunk_group(
                    B - 1, NIC - 1, st_last,
                    after_head=lambda hh: outproj_tt(0, hh, cct0))
                for k in range(1, NCC):
                    cct = load_cct(k)
                    for tt in range(IC // P):
                        outproj_tt(k, tt, cct)

    nc.compile()
    return nc


def kernel(x, wq, wk, wv, wo, freqs_cos, freqs_sin, mask, start_pos=0,
           **_ignored):
    global LAST_RESULTS
    bf = ml_dtypes.bfloat16
    mask = np.asarray(mask, np.float32)
    kinds, blocks = _classify_mask(mask)
    key = kinds.tobytes() + bytes([len(blocks)])
    nc = _GRAPH_CACHE.get(key)
    if nc is None:
        nc = _build(kinds, len(blocks))
        _GRAPH_CACHE[key] = nc

    xt_np = np.ascontiguousarray(
        np.asarray(x, np.float32).reshape(NCH, TCH, ND, P)
        .transpose(0, 3, 2, 1).reshape(NCH, P, ND * TCH)).astype(bf)

    def wtile(w):
        return np.ascontiguousarray(
            w.reshape(4, 8, P, EL).transpose(0, 2, 1, 3)
            .reshape(4, P, 8 * EL))

    def cs_tile(f):
        # [S, EL//2] -> [P, (S//P)*(EL//2)] with row p holding position
        # tiles n*128+p contiguously
        r = np.tile(np.asarray(f, np.float32), (1, HLOC))
        return np.ascontiguousarray(
            r.reshape(S // P, P, EL // 2).transpose(1, 0, 2)
            .reshape(P, (S // P) * (EL // 2))).astype(bf)

    cos_r = cs_tile(freqs_cos)
    sin_r = cs_tile(freqs_sin)
    if blocks:
        # multiplicative post-exp factors: exp(mask * SCALE), transposed
        # to the [keys, queries] score layout
        mb_np = np.ascontiguousarray(np.stack(
            [np.exp(np.clip(b.T, -80.0 / SCALE, 80.0 / SCALE)
                    .astype(np.float64) * SCALE).astype(np.float32)
             for b in blocks])).astype(bf)
    else:
        mb_np = np.zeros((1, P, P), bf)

    in_maps = []
    for c in range(NCORES):
        hs = slice(c * HLOC, (c + 1) * HLOC)
        wq_c = wtile(
            np.asarray(wq, np.float32)[hs].reshape(EL, D).T).astype(bf)
        wk_c = wtile(
            np.asarray(wk, np.float32)[hs].reshape(EL, D).T).astype(bf)
        wv_c = wtile(
            np.asarray(wv, np.float32)[hs].reshape(EL, D).T).astype(bf)
        wo_c = np.ascontiguousarray(
            np.asarray(wo, np.float32)[c * EL:(c + 1) * EL, :].T
            .reshape(ND, P, EL).transpose(1, 0, 2)
            .reshape(P, ND * EL)).astype(bf)
        in_maps.append({
            "xt": xt_np, "wqt": wq_c, "wkt": wk_c, "wvt": wv_c, "wot": wo_c,
            "cosr": cos_r, "sinr": sin_r, "mblk": mb_np,
        })

    res = run_bass_kernel_spmd(nc, in_maps, core_ids=list(range(NCORES)))
    LAST_RESULTS = res
    outs = [res.results[c]["out"] for c in range(NCORES)]
    full = np.concatenate(outs, axis=1).astype(np.float32)
    return full.reshape(B, S, D)
